# revision 8
# baseline (speedup 1.0000x reference)
"""Trainium2 Bass kernel for nn_ChannelAttention (channel-attention over [B,C,T]).

Math (per sample b):
    m[t]   = mean_c x[c,t]
    cov    = (x-m) @ (x-m)^T           (contract over t)
    denom  = sum_t var_unbiased[t]
    pcc    = cov / denom
    q = pcc @ Wq^T + qb ; k = pcc @ Wk^T + kb
    scores = q @ k^T / sqrt(C)
    attn   = softmax(scores, axis=-1)
    out    = attn @ x

Key identities used (avoid any per-t stats / transposes of big tensors):
    G   = x @ x^T (uncentered Gram, contracted over t; computed from a
          bf16 DMA-transposed copy of x whose t-permutation is irrelevant)
    u   = G @ 1 / C            (row sums)
    S   = (1^T G 1) / C^2
    cov = G - u x 1 - 1 x u + S        (rank-1 corrections via K=1 matmuls)
    denom = (trace(G) - C*S) / (C-1)
    scores*sqrt(C) = pcc A pcc + (pcc rk) 1^T + 1 (rq^T pcc) + s0
       with A = Wq^T Wk, rk = Wq^T kb, rq = Wk^T qb, s0 = qb.kb
    scoresT*sqrt(C) = pcc A^T pcc + (pcc rq) 1^T + 1 (rk^T pcc) + s0
    out = diag(1/rowsum(E)) (E^T)^T x with E = exp(scores/sqrt(C)),
       E^T = exp(scoresT/sqrt(C)) computed directly (pcc is symmetric).
Softmax max-shift is skipped: |scores/sqrt(C)| << 1 for this problem's data.
"""

import numpy as np
from contextlib import ExitStack

P = 128  # SBUF partitions


def build_nc(B_loc=2, C=256, T=8192):
    import concourse.bass as bass
    import concourse.tile as tile
    from concourse import bacc, mybir

    f32 = mybir.dt.float32
    bf16 = mybir.dt.bfloat16
    f32r = mybir.dt.float32r
    EXP = mybir.ActivationFunctionType.Exp
    CPY = mybir.ActivationFunctionType.Copy
    MULT = mybir.AluOpType.mult
    ADD = mybir.AluOpType.add
    X = mybir.AxisListType.X

    CG = C // P      # channel groups (2)
    TG = T // P      # t-chunks for the Gram (64)
    TN = 512         # mm2 moving tile
    NT = T // TN     # mm2 t-tiles (16)
    E = 32           # q/k feature dim
    SCL = 1.0 / (C ** 0.5)

    nc = bacc.Bacc("TRN2", target_bir_lowering=False, debug=False)
    x_d = nc.dram_tensor("x", [B_loc, C, T], f32, kind="ExternalInput").ap()
    qw_d = nc.dram_tensor("q_w", [E, C], f32, kind="ExternalInput").ap()
    qb_d = nc.dram_tensor("q_b", [E], f32, kind="ExternalInput").ap()
    kw_d = nc.dram_tensor("k_w", [E, C], f32, kind="ExternalInput").ap()
    kb_d = nc.dram_tensor("k_b", [E], f32, kind="ExternalInput").ap()
    out_d = nc.dram_tensor("out", [B_loc, C, T], f32, kind="ExternalOutput").ap()
    attn_d = nc.dram_tensor("attn", [B_loc, C, C], f32, kind="ExternalOutput").ap()

    with ExitStack() as ctx:
        tc = ctx.enter_context(tile.TileContext(nc))

        const = ctx.enter_context(tc.tile_pool(name="const", bufs=1))
        xn_pool = ctx.enter_context(tc.tile_pool(name="xn", bufs=2))
        xb_pool = ctx.enter_context(tc.tile_pool(name="xb", bufs=1))
        xt_pool = ctx.enter_context(tc.tile_pool(name="xt", bufs=1))
        small = ctx.enter_context(tc.tile_pool(name="small", bufs=2))
        med = ctx.enter_context(tc.tile_pool(name="med", bufs=1))
        outp = ctx.enter_context(tc.tile_pool(name="outp", bufs=3))
        ps_big = ctx.enter_context(tc.tile_pool(name="ps_big", bufs=4, space="PSUM"))
        ps_small = ctx.enter_context(tc.tile_pool(name="ps_small", bufs=2, space="PSUM"))
        ps_out = ctx.enter_context(tc.tile_pool(name="ps_out", bufs=2, space="PSUM"))

        # ---------------- one-time setup ----------------
        qw_sb = const.tile([E, C], f32)
        kw_sb = const.tile([E, C], f32)
        qb_sb = const.tile([E, 1], f32)
        kb_sb = const.tile([E, 1], f32)
        nc.sync.dma_start(out=qw_sb[:], in_=qw_d)
        nc.sync.dma_start(out=kw_sb[:], in_=kw_d)
        nc.sync.dma_start(out=qb_sb[:], in_=qb_d.rearrange("(p o) -> p o", o=1))
        nc.sync.dma_start(out=kb_sb[:], in_=kb_d.rearrange("(p o) -> p o", o=1))

        ones_row = const.tile([1, C], f32)      # row of ones (f32)
        ones_rowb = const.tile([1, P], bf16)    # row of ones (bf16)
        ones_col = const.tile([P, 1], f32)      # column of ones
        nc.vector.memset(ones_row[:], 1.0)
        nc.vector.memset(ones_rowb[:], 1.0)
        nc.vector.memset(ones_col[:], 1.0)

        ident = const.tile([P, P], f32)         # 128x128 identity
        ones128 = const.tile([P, P], f32)
        nc.gpsimd.memset(ones128[:], 1.0)
        nc.gpsimd.affine_select(
            out=ident[:], in_=ones128[:], pattern=[[-1, P]],
            compare_op=mybir.AluOpType.is_equal, fill=0.0,
            base=0, channel_multiplier=1,
        )

        # A = Wq^T Wk  (stored natural [d1, d2]), AT = Wk^T Wq
        a_sb = const.tile([P, CG, C], bf16)
        at_sb = const.tile([P, CG, C], bf16)
        for mh in range(CG):
            aps = ps_big.tile([P, C], f32, tag="big", name="aps")
            nc.tensor.matmul(aps[:], lhsT=qw_sb[:, bass.ts(mh, P)], rhs=kw_sb[:],
                             start=True, stop=True)
            nc.vector.tensor_copy(out=a_sb[:, mh, :], in_=aps[:])
            atps = ps_big.tile([P, C], f32, tag="big", name="aps")
            nc.tensor.matmul(atps[:], lhsT=kw_sb[:, bass.ts(mh, P)], rhs=qw_sb[:],
                             start=True, stop=True)
            nc.vector.tensor_copy(out=at_sb[:, mh, :], in_=atps[:])

        # rk = Wq^T kb, rq = Wk^T qb (columns, [d,1] per chunk), s0 = qb.kb
        rk_bf = const.tile([P, CG], bf16)
        rq_bf = const.tile([P, CG], bf16)
        for mh in range(CG):
            rkps = ps_small.tile([P, 1], f32, tag="ps_s", name="sps")
            nc.tensor.matmul(rkps[:], lhsT=qw_sb[:, bass.ts(mh, P)], rhs=kb_sb[:],
                             start=True, stop=True)
            nc.vector.tensor_copy(out=rk_bf[:, mh:mh + 1], in_=rkps[:])
            rqps = ps_small.tile([P, 1], f32, tag="ps_s", name="sps")
            nc.tensor.matmul(rqps[:], lhsT=kw_sb[:, bass.ts(mh, P)], rhs=qb_sb[:],
                             start=True, stop=True)
            nc.vector.tensor_copy(out=rq_bf[:, mh:mh + 1], in_=rqps[:])
        s0_ps = ps_small.tile([1, 1], f32, tag="ps_s", name="sps")
        nc.tensor.matmul(s0_ps[:], lhsT=qb_sb[:], rhs=kb_sb[:], start=True, stop=True)
        s0_sb = const.tile([1, 1], f32)
        nc.vector.tensor_copy(out=s0_sb[:], in_=s0_ps[:])

        # ---------------- per-sample pipeline ----------------
        for b in range(B_loc):
            # load x natural (f32), cast to bf16, DMA-transpose to [t, c] layout
            xn = xn_pool.tile([P, CG, T], f32r)
            for cg in range(CG):
                nc.sync.dma_start(out=xn[:, cg, :],
                                  in_=x_d[b, bass.ts(cg, P), :].bitcast(f32r))
            xt = xt_pool.tile([P, TG, C], bf16)
            for cg in range(CG):
                xb = xb_pool.tile([P, T], bf16)
                nc.gpsimd.tensor_copy(out=xb[:], in_=xn[:, cg, :])
                # out[p, g, c] = xb[c, g*128+p]  (any consistent bijection works)
                nc.sync.dma_start(out=xt[:, :, bass.ts(cg, P)], in_=xb[:],
                                  transpose=True)

            # Gram: G[mh] = sum_g xt_g[:, mh-slice]^T @ xt_g   -> [c-chunk, d]
            G = []
            for mh in range(CG):
                G.append(ps_big.tile([P, C], f32, tag="big", name=f"G{mh}"))
            for g in range(TG):
                for mh in range(CG):
                    nc.tensor.matmul(G[mh][:], lhsT=xt[:, g, bass.ts(mh, P)],
                                     rhs=xt[:, g, :], start=(g == 0),
                                     stop=(g == TG - 1))

            # row sums and diag of G
            stats = small.tile([P, 4], f32, tag="stats")
            junk = small.tile([P, P], f32, tag="junk")
            for mh in range(CG):
                nc.vector.reduce_sum(out=stats[:, mh:mh + 1], in_=G[mh][:], axis=X)
            for mh in range(CG):
                nc.vector.scalar_tensor_tensor(
                    out=junk[:], in0=G[mh][:, bass.ts(mh, P)], scalar=1.0,
                    in1=ident[:], op0=MULT, op1=MULT,
                    accum_out=stats[:, 2 + mh:3 + mh])
            scal4 = ps_small.tile([1, 4], f32, tag="ps_s")
            nc.tensor.matmul(scal4[:], lhsT=ones_col[:], rhs=stats[:, 0:4],
                             start=True, stop=True)
            # us = sum_cd G ; trace ; denom = (trace - us/C)/(C-1) ; S = us/C^2
            sc = small.tile([1, 8], f32, tag="sc")
            nc.vector.tensor_copy(out=sc[:, 0:4], in_=scal4[:])
            nc.vector.tensor_add(sc[:, 4:5], sc[:, 0:1], sc[:, 1:2])   # us
            nc.vector.tensor_add(sc[:, 5:6], sc[:, 2:3], sc[:, 3:4])   # trace
            # sc6 = us * (-1/(C*(C-1)));  den = trace/(C-1) + sc6
            nc.vector.tensor_scalar(out=sc[:, 6:7], in0=sc[:, 4:5],
                                    scalar1=-1.0 / (C * (C - 1.0)), scalar2=None,
                                    op0=MULT)
            nc.vector.scalar_tensor_tensor(out=sc[:, 7:8], in0=sc[:, 5:6],
                                           scalar=1.0 / (C - 1.0), in1=sc[:, 6:7],
                                           op0=MULT, op1=ADD)
            invd = small.tile([1, 1], f32, tag="invd")
            nc.vector.reciprocal(out=invd[:], in_=sc[:, 7:8])
            S_sb = small.tile([1, 1], f32, tag="S_sb")
            nc.vector.tensor_scalar(out=S_sb[:], in0=sc[:, 4:5],
                                    scalar1=1.0 / (C * C), scalar2=None, op0=MULT)

            # u columns -> u row (via PE transpose)
            ucol = small.tile([P, CG], f32, tag="ucol")
            nc.vector.tensor_scalar(out=ucol[:], in0=stats[:, 0:2],
                                    scalar1=1.0 / C, scalar2=None, op0=MULT)
            urow_ps = ps_small.tile([1, C], f32, tag="ps_s")
            for mh in range(CG):
                nc.tensor.transpose(urow_ps[0:1, bass.ts(mh, P)],
                                    ucol[:, mh:mh + 1], ident[:])
            negu = small.tile([1, C], f32, tag="negu")
            smu = small.tile([1, C], f32, tag="smu")
            nc.vector.tensor_scalar(out=negu[:], in0=urow_ps[:], scalar1=-1.0,
                                    scalar2=None, op0=MULT)
            nc.vector.tensor_scalar(out=smu[:], in0=urow_ps[:], scalar1=-1.0,
                                    scalar2=S_sb[:], op0=MULT, op1=ADD)

            # rank-1 corrections: G += (-u) x 1 + 1 x (S - u)  => G becomes cov
            for mh in range(CG):
                nc.tensor.matmul(G[mh][:], lhsT=negu[0:1, bass.ts(mh, P)],
                                 rhs=ones_row[:], start=False, stop=False,
                                 skip_group_check=True)
                nc.tensor.matmul(G[mh][:], lhsT=ones_row[0:1, 0:P], rhs=smu[:],
                                 start=False, stop=True,
                                 skip_group_check=True)

            # invd broadcast to a [P,1] column, pcc = cov * invd (bf16)
            invcol_ps = ps_small.tile([P, 1], f32, tag="ps_s")
            nc.tensor.matmul(invcol_ps[:], lhsT=ones_row[0:1, 0:P], rhs=invd[:],
                             start=True, stop=True)
            invcol = small.tile([P, 1], f32, tag="invcol_sb")
            nc.vector.tensor_copy(out=invcol[:], in_=invcol_ps[:])
            pcc = med.tile([P, CG, C], bf16, tag="pcc")
            for mh in range(CG):
                nc.vector.tensor_scalar(out=pcc[:, mh, :], in0=G[mh][:],
                                        scalar1=invcol[:], scalar2=None, op0=MULT)

            # P2a = A @ pcc, P2b = A^T @ pcc
            p2a = med.tile([P, CG, C], bf16, tag="p2a")
            p2b = med.tile([P, CG, C], bf16, tag="p2b")
            for mh in range(CG):
                pa = ps_big.tile([P, C], f32, tag="big")
                pb = ps_big.tile([P, C], f32, tag="big")
                for kh in range(CG):
                    nc.tensor.matmul(pa[:], lhsT=at_sb[:, kh, bass.ts(mh, P)],
                                     rhs=pcc[:, kh, :], start=(kh == 0),
                                     stop=(kh == CG - 1))
                    nc.tensor.matmul(pb[:], lhsT=a_sb[:, kh, bass.ts(mh, P)],
                                     rhs=pcc[:, kh, :], start=(kh == 0),
                                     stop=(kh == CG - 1))
                nc.vector.tensor_copy(out=p2a[:, mh, :], in_=pa[:])
                nc.vector.tensor_copy(out=p2b[:, mh, :], in_=pb[:])

            # bias columns: (pcc @ rk + s0)/sqrt(C) for scores, rq for scoresT
            bias_n = small.tile([P, CG], f32, tag="bias_n")
            bias_t = small.tile([P, CG], f32, tag="bias_t")
            for mh in range(CG):
                un = ps_small.tile([P, 1], f32, tag="ps_s")
                for kh in range(CG):
                    nc.tensor.matmul(un[:], lhsT=pcc[:, kh, bass.ts(mh, P)],
                                     rhs=rk_bf[:, kh:kh + 1], start=(kh == 0),
                                     stop=False)
                nc.tensor.matmul(un[:], lhsT=ones_row[0:1, 0:P], rhs=s0_sb[:],
                                 start=False, stop=True)
                nc.vector.tensor_scalar(out=bias_n[:, mh:mh + 1], in0=un[:],
                                        scalar1=SCL, scalar2=None, op0=MULT)
                ut = ps_small.tile([P, 1], f32, tag="ps_s")
                for kh in range(CG):
                    nc.tensor.matmul(ut[:], lhsT=pcc[:, kh, bass.ts(mh, P)],
                                     rhs=rq_bf[:, kh:kh + 1], start=(kh == 0),
                                     stop=False)
                nc.tensor.matmul(ut[:], lhsT=ones_row[0:1, 0:P], rhs=s0_sb[:],
                                 start=False, stop=True)
                nc.vector.tensor_scalar(out=bias_t[:, mh:mh + 1], in0=ut[:],
                                        scalar1=SCL, scalar2=None, op0=MULT)

            # row terms: rq^T pcc and rk^T pcc (bf16 rows)
            rqrow = small.tile([1, C], bf16, tag="rqrow")
            rkrow = small.tile([1, C], bf16, tag="rkrow")
            rr = ps_small.tile([1, C], f32, tag="ps_s")
            for kh in range(CG):
                nc.tensor.matmul(rr[:], lhsT=rq_bf[:, kh:kh + 1], rhs=pcc[:, kh, :],
                                 start=(kh == 0), stop=(kh == CG - 1))
            nc.vector.tensor_copy(out=rqrow[:], in_=rr[:])
            rr2 = ps_small.tile([1, C], f32, tag="ps_s")
            for kh in range(CG):
                nc.tensor.matmul(rr2[:], lhsT=rk_bf[:, kh:kh + 1], rhs=pcc[:, kh, :],
                                 start=(kh == 0), stop=(kh == CG - 1))
            nc.vector.tensor_copy(out=rkrow[:], in_=rr2[:])

            # scores (natural + transposed), exp, rowsums
            e_n = med.tile([P, CG, C], f32, tag="e_n")
            e_t = med.tile([P, CG, C], f32r, tag="e_t")
            dsum = small.tile([P, CG], f32, tag="dsum")
            for mh in range(CG):
                s1 = ps_big.tile([P, C], f32, tag="big")
                for kh in range(CG):
                    nc.tensor.matmul(s1[:], lhsT=pcc[:, kh, bass.ts(mh, P)],
                                     rhs=p2a[:, kh, :], start=(kh == 0), stop=False)
                nc.tensor.matmul(s1[:], lhsT=ones_rowb[:], rhs=rqrow[:],
                                 start=False, stop=True)
                nc.scalar.activation(out=e_n[:, mh, :], in_=s1[:], func=EXP,
                                     bias=bias_n[:, mh:mh + 1], scale=SCL,
                                     accum_out=dsum[:, mh:mh + 1])
                s2 = ps_big.tile([P, C], f32, tag="big")
                for kh in range(CG):
                    nc.tensor.matmul(s2[:], lhsT=pcc[:, kh, bass.ts(mh, P)],
                                     rhs=p2b[:, kh, :], start=(kh == 0), stop=False)
                nc.tensor.matmul(s2[:], lhsT=ones_rowb[:], rhs=rkrow[:],
                                 start=False, stop=True)
                nc.scalar.activation(out=e_t[:, mh, :], in_=s2[:], func=EXP,
                                     bias=bias_t[:, mh:mh + 1], scale=SCL)
            invD = small.tile([P, CG], f32, tag="invD")
            nc.vector.reciprocal(out=invD[:], in_=dsum[:])

            # attn output
            attn_sb = med.tile([P, CG, C], f32, tag="attn_sb")
            for mh in range(CG):
                nc.vector.tensor_scalar(out=attn_sb[:, mh, :], in0=e_n[:, mh, :],
                                        scalar1=invD[:, mh:mh + 1], scalar2=None,
                                        op0=MULT)
                nc.sync.dma_start(out=attn_d[b, bass.ts(mh, P), :],
                                  in_=attn_sb[:, mh, :])

            # out = diag(invD) E x : lhsT = E^T (f32r), rhs = xn (f32r)
            for cg in range(CG):
                for tt in range(NT):
                    ot = ps_out.tile([P, TN], f32, tag="ot")
                    for kh in range(CG):
                        nc.tensor.matmul(
                            ot[:], lhsT=e_t[:, kh, bass.ts(cg, P)],
                            rhs=xn[:, kh, bass.ts(tt, TN)],
                            start=(kh == 0), stop=(kh == CG - 1))
                    osb = outp.tile([P, TN], f32, tag="osb")
                    nc.scalar.activation(out=osb[:], in_=ot[:], func=CPY,
                                         bias=0.0, scale=invD[:, cg:cg + 1])
                    nc.sync.dma_start(out=out_d[b, bass.ts(cg, P), bass.ts(tt, TN)],
                                      in_=osb[:])

    nc.compile()
    return nc


_NC_CACHE = {}
TRACE = False          # set True to capture an NTFF profile on the next kernel() call
LAST_RESULTS = None    # BassKernelResults of the last kernel() call


def _get_nc(B_loc, C, T):
    key = (B_loc, C, T)
    if key not in _NC_CACHE:
        _NC_CACHE[key] = build_nc(B_loc, C, T)
    return _NC_CACHE[key]


def kernel(x, q_w, q_b, k_w, k_b):
    from concourse.bass_utils import run_bass_kernel_spmd

    x = np.ascontiguousarray(np.asarray(x, dtype=np.float32))
    q_w = np.ascontiguousarray(np.asarray(q_w, dtype=np.float32))
    q_b = np.ascontiguousarray(np.asarray(q_b, dtype=np.float32))
    k_w = np.ascontiguousarray(np.asarray(k_w, dtype=np.float32))
    k_b = np.ascontiguousarray(np.asarray(k_b, dtype=np.float32))

    B, C, T = x.shape
    n_cores = 8
    assert B % n_cores == 0
    B_loc = B // n_cores

    nc = _get_nc(B_loc, C, T)
    in_maps = [
        {"x": x[i * B_loc:(i + 1) * B_loc], "q_w": q_w, "q_b": q_b,
         "k_w": k_w, "k_b": k_b}
        for i in range(n_cores)
    ]
    res = run_bass_kernel_spmd(nc, in_maps, core_ids=list(range(n_cores)),
                               trace=TRACE)
    global LAST_RESULTS
    LAST_RESULTS = res
    out = np.concatenate([r["out"] for r in res.results], axis=0)
    attn = np.concatenate([r["attn"] for r in res.results], axis=0)
    return out, attn
